# revision 6
# baseline (speedup 1.0000x reference)
"""Causal self-attention (QK-RMSNorm + rotary + value-embed blend) on 8 trn2 cores.

Sharding: 8 cores = 4 batches x 2 head-halves (8 heads each).
Host pre-transposes/casts inputs to fp16; device computes a per-core output
partial [1024, 1024] fp32 (output projection contracted over its 512 hdim
columns); host sums the two partials per batch.

Math notes:
 - scores are bounded (|s| <= 0.12*64 = 7.68 after QK RMS-norm), so softmax
   is computed without max-subtraction: exp then divide by sum.
 - denominator comes from an extra all-ones column appended to V.
 - causal mask: matmuls restricted to q >= kt*128; the single diagonal
   128-block per (head, kt) is masked multiplicatively after exp.
 - rsqrt for RMS-norm is exp(-0.5*ln(x)) so the only ACT tables used are
   ln/exp (one table set, no thrash with the attention exp).
"""

import sys

if "/opt/trn_rl_repo" not in sys.path:
    sys.path.insert(0, "/opt/trn_rl_repo")

import numpy as np

import concourse.bass as bass
import concourse.mybir as mybir
import concourse.tile as tile
from concourse.bass import ds, ts

P = 128
B, T, D = 4, 1024, 1024
H, DH = 16, 64
H8 = 8            # heads per core
ATTN_SCALE = 0.12
EPS = 1e-6
N_CORES = 8
TT_N = T // P     # 8 t-tiles
DT_N = D // P     # 8 d-tiles (contraction)
KT_N = T // P     # 8 kpos tiles
RT_N = (H8 * DH) // P  # 4 row-tiles of Q^T/K^T
JT_N = (H8 * DH) // P  # 4 j-tiles for out-proj contraction

f16 = mybir.dt.float16
f32 = mybir.dt.float32


def split_sync_waits(nc, max_waits=1):
    """This container's walrus rejects >1 sync-wait per instruction; spill
    extra waits onto preceding NoOps on the same engine."""
    n = 0
    for fn in nc.m.functions:
        for bb in fn.blocks:
            new_insts = []
            for inst in bb.instructions:
                si = getattr(inst, "sync_info", None)
                if si is not None and si.on_wait and len(si.on_wait) > max_waits:
                    waits = list(si.on_wait)
                    extra, keep = waits[:-max_waits], waits[-max_waits:]
                    for w in extra:
                        nop = mybir.InstNoOp(
                            name=nc.get_next_instruction_name(),
                            sync_info=mybir.SyncInfo(on_wait=[w], on_update=[]),
                            bass_nofuse=True,
                            engine=inst.engine,
                        )
                        nc.register_instruction(nop)
                        new_insts.append(nop)
                        n += 1
                    si.on_wait = keep
                new_insts.append(inst)
            bb.instructions[:] = new_insts
    return n


def build_nc(reps=1):
    nc = bass.Bass()

    xT = nc.declare_dram_parameter("xT", [D, T], f16, isOutput=False)
    wqT = nc.declare_dram_parameter("wqT", [D, H8 * DH], f16, isOutput=False)
    wkT = nc.declare_dram_parameter("wkT", [D, H8 * DH], f16, isOutput=False)
    wvT = nc.declare_dram_parameter("wvT", [D, H8 * DH], f16, isOutput=False)
    woT = nc.declare_dram_parameter("woT", [H8 * DH, D], f16, isOutput=False)
    ve = nc.declare_dram_parameter("ve", [T, H8 * DH], f16, isOutput=False)
    cosd = nc.declare_dram_parameter("cosd", [T, 16], f16, isOutput=False)
    sind = nc.declare_dram_parameter("sind", [T, 32], f16, isOutput=False)
    trim = nc.declare_dram_parameter("trim", [P, P], f16, isOutput=False)
    out = nc.declare_dram_parameter("out", [T, D], f32, isOutput=True)

    xT_v = xT.rearrange("(dt p) t -> p dt t", p=P)
    wq_v = wqT.rearrange("(dt p) r -> p dt r", p=P)
    wk_v = wkT.rearrange("(dt p) r -> p dt r", p=P)
    wv_v = wvT.rearrange("(dt p) r -> p dt r", p=P)
    wo_v = woT.rearrange("(jt p) i -> p jt i", p=P)
    ve_v = ve.rearrange("(tt p) r -> p tt r", p=P)
    cos_v = cosd.rearrange("(tt p) e -> p tt e", p=P)
    sin_v = sind.rearrange("(tt p) e -> p tt e", p=P)
    out_v = out.rearrange("(tt p) i -> p tt i", p=P)

    with tile.TileContext(nc) as tc:
        import contextlib

        with contextlib.ExitStack() as ctx:
            const = ctx.enter_context(tc.tile_pool(name="const", bufs=1))
            big = ctx.enter_context(tc.tile_pool(name="big", bufs=1))
            dram = ctx.enter_context(tc.tile_pool(name="dram", bufs=1, space="DRAM"))

            # -------- constant / persistent tiles --------
            wq_sb = const.tile([P, DT_N, H8 * DH], f16)
            wk_sb = const.tile([P, DT_N, H8 * DH], f16)
            wv_sb = const.tile([P, DT_N, H8 * DH], f16)
            wo_sb = const.tile([P, JT_N, D], f16)
            ve_sb = const.tile([P, TT_N, H8 * DH], f16)
            cos_sb = const.tile([P, TT_N, 16], f16)
            sin_sb = const.tile([P, TT_N, 32], f16)
            trim_sb = const.tile([P, P], f16)
            xT_sb = big.tile([P, DT_N, T], f16)

            nc.sync.dma_start(out=xT_sb[:], in_=xT_v[:])
            nc.sync.dma_start(out=wq_sb[:], in_=wq_v[:])
            nc.sync.dma_start(out=wk_sb[:], in_=wk_v[:])
            nc.sync.dma_start(out=wv_sb[:], in_=wv_v[:])
            nc.sync.dma_start(out=wo_sb[:], in_=wo_v[:])
            nc.sync.dma_start(out=ve_sb[:], in_=ve_v[:])
            nc.sync.dma_start(out=cos_sb[:], in_=cos_v[:])
            nc.sync.dma_start(out=sin_sb[:], in_=sin_v[:])
            nc.sync.dma_start(out=trim_sb[:], in_=trim[:])

            # persistent activations
            Qn = big.tile([P, TT_N, H8 * DH], f16)   # q normalized+rotated, [t, r]
            Kn = big.tile([P, TT_N, H8 * DH], f16)
            vp = big.tile([P, KT_N, H8, DH + 1], f16)  # V blended + ones col
            y16 = big.tile([P, TT_N, H8 * DH], f16)
            QT_sb = big.tile([P, RT_N, T], f16)      # [r, t]
            KT_sb = big.tile([P, RT_N, T], f16)
            yT_sb = big.tile([P, JT_N, T], f16)

            nc.vector.memset(vp[:, :, :, DH], 1.0)
            eps_sb = const.tile([P, 1], f32)
            nc.vector.memset(eps_sb[:], EPS)

            for _rep in range(reps):
                # ================= Phase B: QKV projection + norm + rotary ===========
                with tc.tile_pool(name="projpsum", bufs=2, space="PSUM") as psB, \
                     tc.tile_pool(name="stat", bufs=3) as statp:
                    for tt in range(TT_N):
                        pq = psB.tile([P, H8 * DH], f32, tag="pq")
                        pk = psB.tile([P, H8 * DH], f32, tag="pk")
                        pv = psB.tile([P, H8 * DH], f32, tag="pv")
                        for dt in range(DT_N):
                            lx = xT_sb[:, dt, ts(tt, P)]
                            st, sp = dt == 0, dt == DT_N - 1
                            nc.tensor.matmul(pq[:], lx, wq_sb[:, dt, :], start=st, stop=sp)
                            nc.tensor.matmul(pk[:], lx, wk_sb[:, dt, :], start=st, stop=sp)
                            nc.tensor.matmul(pv[:], lx, wv_sb[:, dt, :], start=st, stop=sp)

                        # V blend: v = lambda0*v (folded in weights) + ve (prescaled)
                        nc.vector.tensor_tensor(
                            vp[:, tt, :, 0:DH],
                            pv.rearrange("p (h e) -> p h e", h=H8),
                            ve_sb[:, tt].rearrange("p (h e) -> p h e", h=H8),
                            mybir.AluOpType.add,
                        )

                        # RMS stats for q,k: mean(x^2) then rsqrt via exp(-0.5 ln)
                        for which, psrc, dst in (("q", pq, Qn), ("k", pk, Kn)):
                            sq = statp.tile([P, H8 * DH], f16, tag="sq")
                            nc.scalar.square(sq[:], psrc[:])
                            ms = statp.tile([P, H8], f32, tag="ms")
                            nc.vector.reduce_sum(
                                ms[:],
                                sq.rearrange("p (h e) -> p h e", h=H8),
                                axis=mybir.AxisListType.X,
                            )
                            lnv = statp.tile([P, H8], f32, tag="lnv")
                            nc.scalar.activation(
                                lnv[:], ms[:], mybir.ActivationFunctionType.Ln,
                                bias=eps_sb[:], scale=1.0 / DH,
                            )
                            scl = statp.tile([P, H8], f32, tag="scl")
                            nc.scalar.activation(
                                scl[:], lnv[:], mybir.ActivationFunctionType.Exp,
                                scale=-0.5,
                            )
                            # fused normalize + copy psum->sbuf fp16
                            nc.vector.tensor_tensor(
                                dst[:, tt].rearrange("p (h e) -> p h e", h=H8),
                                psrc.rearrange("p (h e) -> p h e", h=H8),
                                scl[:, :, None].to_broadcast((P, H8, DH)),
                                mybir.AluOpType.mult,
                            )

                        # rotary on rotating cols (e<16 of each half), in fp16
                        for dst in (Qn, Kn):
                            rot = dst[:, tt].rearrange(
                                "p (h half eh e) -> p h half eh e", h=H8, half=2, eh=2
                            )[:, :, :, 0, :]  # [P, H8, 2, 16]
                            qsw = statp.tile([P, H8, 2, 16], f16, tag="qsw")
                            nc.vector.tensor_copy(qsw[:, :, 0, :], rot[:, :, 1, :])
                            nc.vector.tensor_copy(qsw[:, :, 1, :], rot[:, :, 0, :])
                            t1 = statp.tile([P, H8, 2, 16], f16, tag="t1")
                            nc.vector.tensor_tensor(
                                t1[:], rot,
                                cos_sb[:, tt][:, None, None, :].to_broadcast((P, H8, 2, 16)),
                                mybir.AluOpType.mult,
                            )
                            t2 = statp.tile([P, H8, 2, 16], f16, tag="t2")
                            nc.vector.tensor_tensor(
                                t2[:], qsw[:],
                                sin_sb[:, tt].rearrange("p (half e) -> p half e", half=2)[
                                    :, None, :, :
                                ].to_broadcast((P, H8, 2, 16)),
                                mybir.AluOpType.mult,
                            )
                            nc.vector.tensor_tensor(rot, t1[:], t2[:], mybir.AluOpType.add)

                # ====== Phase C: transpose Q,K (SBUF->SBUF xbar, per 128x128) ======
                for rb in range(RT_N):
                    for tt in range(TT_N):
                        nc.sync.dma_start_transpose(
                            QT_sb[:, rb, ts(tt, P)], Qn[:, tt, ts(rb, P)]
                        )
                        nc.sync.dma_start_transpose(
                            KT_sb[:, rb, ts(tt, P)], Kn[:, tt, ts(rb, P)]
                        )

                # ================= Phase D: attention per head-pair ================
                with tc.tile_pool(name="attpsum", bufs=2, space="PSUM") as psD, \
                     tc.tile_pool(name="etpool", bufs=2) as etp, \
                     tc.tile_pool(name="avtmp", bufs=3) as avt:
                    for g in range(H8 // 2):
                        ets = []
                        for hb in range(2):
                            et = etp.tile([P, KT_N, T], f16, tag=f"et{hb}")
                            ets.append(et)
                        # ---- scores^T + exp + mask ----
                        for kt in range(KT_N):
                            qlo = kt * P
                            for hb in range(2):
                                h = 2 * g + hb
                                lo, hi = hb * 64, hb * 64 + 64
                                pst = psD.tile([P, T], f32, tag=f"st{hb}")
                                for qh in range(2):
                                    qs = max(qh * 512, qlo)
                                    qe = (qh + 1) * 512
                                    if qs >= qe:
                                        continue
                                    nc.tensor.matmul(
                                        pst[:, ds(qs, qe - qs)],
                                        KT_sb[lo:hi, g, ts(kt, P)],
                                        QT_sb[lo:hi, g, ds(qs, qe - qs)],
                                        start=True, stop=True,
                                    )
                                nc.scalar.activation(
                                    ets[hb][:, kt, ds(qlo, T - qlo)],
                                    pst[:, ds(qlo, T - qlo)],
                                    mybir.ActivationFunctionType.Exp,
                                    scale=ATTN_SCALE,
                                )
                            # mask the diagonal block
                            for hb in range(2):
                                nc.vector.tensor_tensor(
                                    ets[hb][:, kt, ds(qlo, P)],
                                    ets[hb][:, kt, ds(qlo, P)],
                                    trim_sb[:],
                                    mybir.AluOpType.mult,
                                )
                        # ---- AV + divide ----
                        for qt in range(TT_N):
                            pav = psD.tile([P, 130], f32, tag="av")
                            for hb in range(2):
                                h = 2 * g + hb
                                for kt in range(qt + 1):
                                    nc.tensor.matmul(
                                        pav[:, ds(hb * 65, 65)],
                                        ets[hb][:, kt, ts(qt, P)],
                                        vp[:, kt, h, :],
                                        start=(kt == 0), stop=(kt == qt),
                                    )
                            pavv = pav.rearrange("p (h c) -> p h c", h=2)
                            r = avt.tile([P, 2], f32, tag="r")
                            nc.vector.reciprocal(r[:], pavv[:, :, DH : DH + 1])
                            nc.vector.tensor_tensor(
                                y16[:, qt, ds(2 * g * DH, 2 * DH)].rearrange(
                                    "p (h e) -> p h e", h=2
                                ),
                                pavv[:, :, 0:DH],
                                r[:, :, None].to_broadcast((P, 2, DH)),
                                mybir.AluOpType.mult,
                            )

                # ================= Phase E: transpose y via DRAM ==================
                nc.sync.dma_start(
                    out=y_dr.rearrange("(tt p) r -> p tt r", p=P), in_=y16[:]
                )
                for rb in range(JT_N):
                    nc.sync.dma_start_transpose(yT_sb[:, rb, :], y_dr[:, ts(rb, P)])

                # ================= Phase F: output projection =====================
                with tc.tile_pool(name="outpsum", bufs=2, space="PSUM") as psF, \
                     tc.tile_pool(name="outstage", bufs=3) as osp:
                    for tt in range(TT_N):
                        for ic in range(2):
                            po = psF.tile([P, 512], f32, tag="po")
                            for jt in range(JT_N):
                                nc.tensor.matmul(
                                    po[:],
                                    yT_sb[:, jt, ts(tt, P)],
                                    wo_sb[:, jt, ds(ic * 512, 512)],
                                    start=(jt == 0), stop=(jt == JT_N - 1),
                                )
                            osb = osp.tile([P, 512], f32, tag="osb")
                            nc.any.tensor_copy(out=osb[:], in_=po[:])
                            nc.sync.dma_start(
                                out=out_v[:, tt, ds(ic * 512, 512)], in_=osb[:]
                            )

    split_sync_waits(nc)
    return nc


def make_core_inputs(x, qkvo_w, value_embeds, lambda_v):
    """Host-side prep: returns list of per-core input dicts (fp16)."""
    x = np.asarray(x)
    qkvo_w = np.asarray(qkvo_w)
    value_embeds = np.asarray(value_embeds)
    lambda_v = np.asarray(lambda_v)

    freq = (1.0 / 1024.0) ** np.linspace(0.0, 1.0, DH // 4, dtype=np.float32)
    theta = np.arange(T, dtype=np.float32)[:, None] * freq[None, :]  # [T, 16]
    cos16 = np.cos(theta).astype(np.float16)
    sin = np.sin(theta).astype(np.float32)
    sinsgn = np.concatenate([sin, -sin], axis=1).astype(np.float16)  # [T, 32]
    trimask = np.triu(np.ones((P, P), dtype=np.float16))  # M[k,q]=1 iff k<=q

    in_maps = []
    for c in range(N_CORES):
        b, hh = c // 2, c % 2
        R = slice(hh * H8 * DH, (hh + 1) * H8 * DH)
        in_maps.append({
            "xT": np.ascontiguousarray(x[b].T).astype(np.float16),
            "wqT": np.ascontiguousarray(qkvo_w[0][R].T).astype(np.float16),
            "wkT": np.ascontiguousarray(qkvo_w[1][R].T).astype(np.float16),
            "wvT": np.ascontiguousarray(
                (lambda_v[0] * qkvo_w[2][R]).T
            ).astype(np.float16),
            "woT": np.ascontiguousarray(qkvo_w[3][:, R].T).astype(np.float16),
            "ve": (lambda_v[1] * value_embeds[:T, R]).astype(np.float16),
            "cosd": cos16,
            "sind": sinsgn,
            "trim": trimask,
        })
    return in_maps


_NC_CACHE = {}


def _get_nc(reps=1):
    if reps not in _NC_CACHE:
        _NC_CACHE[reps] = build_nc(reps)
    return _NC_CACHE[reps]


def kernel(x, qkvo_w, value_embeds, lambda_v):
    from concourse.bass_utils import run_bass_kernel_spmd

    nc = _get_nc()
    in_maps = make_core_inputs(x, qkvo_w, value_embeds, lambda_v)
    res = run_bass_kernel_spmd(nc, in_maps, list(range(N_CORES))).results
    out = np.empty((B, T, D), dtype=np.float32)
    for b in range(B):
        out[b] = res[2 * b]["out"] + res[2 * b + 1]["out"]
    return out


# revision 47
# speedup vs baseline: 868.5086x; 868.5086x over previous
"""Causal self-attention (QK-RMSNorm + rotary + value-embed blend) on 8 trn2 cores.

Sharding: 8 cores = 4 batches x 2 head-halves (8 heads each).
Host pre-transposes/casts inputs to fp16; device computes a per-core output
partial [1024, 1024] fp32 (output projection contracted over its 512 hdim
columns); host sums the two partials per batch.

Per-core kernel structure (pipelined per head-pair g in 0..3):
  proj(g): qkv = x @ W[:, pair-cols]   (fp16 matmuls, fp32 psum)
  norm+rotary on q,k (DVE, fused with psum->sbuf fp16 copy)
  transpose q,k via DMA-xbar 128x128 blocks (fp16)
  scores^T = K^T q (row-tiled pair, contraction=64), exp via ACT from psum
  causal: matmuls restricted to q >= kt*128, diagonal block masked after exp
  AV: y[q,:] = sum_kt E^T[kt]^T @ [V|1]; softmax denominator from the ones col
  y/denom via per-partition reciprocal + broadcast multiply
Then out_partial = y @ woT (contract local 512 hdim cols).

Numerics: scores bounded (|s| <= 7.68) so no max-subtraction needed; rsqrt
computed as exp(-0.5*ln(x)) so ACT only ever uses the ln/exp table set.
"""

import sys

if "/opt/trn_rl_repo" not in sys.path:
    sys.path.insert(0, "/opt/trn_rl_repo")

import numpy as np

import concourse.bass as bass
import concourse.mybir as mybir
import concourse.tile as tile
from concourse.bass import ds, ts

P = 128
B, T, D = 4, 1024, 1024
H, DH = 16, 64
H8 = 8            # heads per core
NG = H8 // 2      # head pairs
ATTN_SCALE = 0.12
EPS = 1e-6
N_CORES = 8
TT_N = T // P     # 8 t-tiles
DT_N = D // P     # 8 d-tiles (contraction)
KT_N = T // P     # 8 kpos tiles
RT_N = (H8 * DH) // P  # 4 row-tiles of Q^T/K^T
JT_N = (H8 * DH) // P  # 4 j-tiles for out-proj contraction

f16 = mybir.dt.float16
f32 = mybir.dt.float32


def split_sync_waits(nc, max_waits=1):
    """This container's walrus rejects >1 sync-wait per instruction; spill
    extra waits onto preceding NoOps on the same engine."""
    n = 0
    for fn in nc.m.functions:
        for bb in fn.blocks:
            new_insts = []
            for inst in bb.instructions:
                si = getattr(inst, "sync_info", None)
                if si is not None and si.on_wait and len(si.on_wait) > max_waits:
                    waits = list(si.on_wait)
                    extra, keep = waits[:-max_waits], waits[-max_waits:]
                    for w in extra:
                        nop = mybir.InstNoOp(
                            name=nc.get_next_instruction_name(),
                            sync_info=mybir.SyncInfo(on_wait=[w], on_update=[]),
                            bass_nofuse=True,
                            engine=inst.engine,
                        )
                        nc.register_instruction(nop)
                        new_insts.append(nop)
                        n += 1
                    si.on_wait = keep
                new_insts.append(inst)
            bb.instructions[:] = new_insts
    return n


def build_nc(reps=1):
    nc = bass.Bass()

    xT = nc.declare_dram_parameter("xT", [D, T], f16, isOutput=False)
    # per-pair contiguous fused qkv weights: [D, pair, (q|k|v)*128]
    wqkv = nc.declare_dram_parameter("wqkv", [D, NG, 384], f16, isOutput=False)
    woT = nc.declare_dram_parameter("woT", [H8 * DH, D], f16, isOutput=False)
    ve = nc.declare_dram_parameter("ve", [T, H8 * DH], f16, isOutput=False)
    cosd = nc.declare_dram_parameter("cosd", [T, 64], f16, isOutput=False)
    sind = nc.declare_dram_parameter("sind", [T, 64], f16, isOutput=False)
    # causal ramp-mask factors: (Am.T @ Bm)[k, q] = -C * max(0, k - q)
    amask = nc.declare_dram_parameter("amask", [P, P], f16, isOutput=False)
    bmask = nc.declare_dram_parameter("bmask", [P, P], f16, isOutput=False)
    out = nc.declare_dram_parameter("out", [T, D], f32, isOutput=True)

    xT_v = xT.rearrange("(dt p) t -> p dt t", p=P)
    wqkv_v = wqkv.rearrange("(dt p) g r -> p dt g r", p=P)
    wo_v = woT.rearrange("(jt p) i -> p jt i", p=P)
    ve_v = ve.rearrange("(tt p) r -> p tt r", p=P)
    cos_v = cosd.rearrange("(tt p) e -> p tt e", p=P)
    sin_v = sind.rearrange("(tt p) e -> p tt e", p=P)
    out_v = out.rearrange("(tt p) i -> p tt i", p=P)

    with tile.TileContext(nc) as tc:
        import contextlib

        with contextlib.ExitStack() as ctx:
            const = ctx.enter_context(tc.tile_pool(name="const", bufs=1))
            big = ctx.enter_context(tc.tile_pool(name="big", bufs=1))

            # -------- persistent tiles --------
            xT_sb = big.tile([P, DT_N, T], f16)
            wqkv_sb = const.tile([P, DT_N, NG, 384], f16)
            wo_sb = const.tile([P, JT_N, D], f16)
            ve_sb = const.tile([P, TT_N, H8 * DH], f16)
            cos_sb = const.tile([P, TT_N, 64], f16)
            sin_sb = const.tile([P, TT_N, 64], f16)
            am_sb = const.tile([P, P], f16)
            bm_sb = const.tile([P, P], f16)

            # loads: pair-0 weights + x first so the pipeline starts early
            nc.sync.dma_start(out=wqkv_sb[:, :, 0, :], in_=wqkv_v[:, :, 0, :])
            for h in range(2):
                hd = ds(h * 4, 4)
                nc.sync.dma_start(out=xT_sb[:, hd, :], in_=xT_v[:, hd, :])
            nc.sync.dma_start(out=ve_sb[:], in_=ve_v[:])
            nc.sync.dma_start(out=cos_sb[:], in_=cos_v[:])
            nc.sync.dma_start(out=sin_sb[:], in_=sin_v[:])
            nc.sync.dma_start(out=am_sb[:], in_=amask[:])
            nc.sync.dma_start(out=bm_sb[:], in_=bmask[:])
            for g in range(1, NG):
                nc.sync.dma_start(
                    out=wqkv_sb[:, :, g, :], in_=wqkv_v[:, :, g, :])
            nc.sync.dma_start(out=wo_sb[:], in_=wo_v[:])

            vp = big.tile([P, KT_N, H8, DH + 1], f16)  # V blended + ones col
            y16 = big.tile([P, TT_N, H8 * DH], f16)
            QT_sb = big.tile([P, RT_N, T], f16)      # [r, t] fp16
            KT_sb = big.tile([P, RT_N, T], f16)
            yT_sb = big.tile([P, JT_N, T], f16)

            nc.vector.memset(vp[:, :, :, DH], 1.0)
            eps_sb = const.tile([P, 1], f32)
            nc.vector.memset(eps_sb[:], EPS)

            for _rep in range(reps):
                with tc.tile_pool(name="projps", bufs=2, space="PSUM") as psB, \
                     tc.tile_pool(name="stps", bufs=1, space="PSUM") as psST, \
                     tc.tile_pool(name="avps", bufs=2, space="PSUM") as psAV, \
                     tc.tile_pool(name="qk", bufs=2) as qkp, \
                     tc.tile_pool(name="etp", bufs=2) as etp, \
                     tc.tile_pool(name="qkdr", bufs=2, space="DRAM") as qkdr, \
                     tc.tile_pool(name="stat", bufs=4) as statp:
                    for g in range(NG):
                        gc = ts(g, P)  # this pair's 128 cols in q/k/v row space
                        QKg = qkp.tile([P, TT_N, 2 * P], f16, tag="qkg")
                        Qg = QKg[:, :, 0:P]
                        Kg = QKg[:, :, P : 2 * P]
                        # ---------- projection (+v blend, raw qk copy) ----------
                        sqg = statp.tile([P, TT_N, 256], f16, tag="sqg")
                        for tt in range(TT_N):
                            pj = psB.tile([P, 384], f32, tag="pqkv")
                            # single psum accumulation group for the whole
                            # bank (q,k,v ranges interleave; per-element
                            # has_written handles first-write-overwrite)
                            for dt in range(DT_N):
                                lx = xT_sb[:, dt, ts(tt, P)]
                                nc.tensor.matmul(
                                    pj[:, 0:128], lx, wqkv_sb[:, dt, g, 0:128],
                                    start=(dt == 0), stop=False)
                                nc.tensor.matmul(
                                    pj[:, 128:256], lx,
                                    wqkv_sb[:, dt, g, 128:256],
                                    start=False, stop=False)
                                nc.tensor.matmul(
                                    pj[:, 256:384], lx,
                                    wqkv_sb[:, dt, g, 256:384],
                                    start=False, stop=(dt == DT_N - 1))
                            # v blend -> vp
                            nc.vector.tensor_tensor(
                                vp[:, tt, 2 * g : 2 * g + 2, 0:DH],
                                pj[:, 256:384].rearrange("p (h e) -> p h e", h=2),
                                ve_sb[:, tt, gc].rearrange("p (h e) -> p h e", h=2),
                                mybir.AluOpType.add,
                            )
                            # raw q,k copy (normalized later, batched)
                            nc.any.tensor_copy(out=QKg[:, tt, :], in_=pj[:, 0:256])
                            nc.scalar.square(sqg[:, tt, :], pj[:, 0:256])
                        # ---------- batched RMS-norm scale ----------
                        ms = statp.tile([P, TT_N, 4], f32, tag="ms")
                        nc.vector.reduce_sum(
                            ms[:],
                            sqg.rearrange("p tt (h e) -> p tt h e", h=4),
                            axis=mybir.AxisListType.X,
                        )
                        lnv = statp.tile([P, TT_N, 4], f32, tag="lnv")
                        nc.scalar.activation(
                            lnv[:], ms[:], mybir.ActivationFunctionType.Ln,
                            bias=eps_sb[:], scale=1.0 / DH,
                        )
                        scl = statp.tile([P, TT_N, 4], f32, tag="scl")
                        nc.scalar.activation(
                            scl[:], lnv[:], mybir.ActivationFunctionType.Exp,
                            scale=-0.5,
                        )
                        nc.vector.tensor_tensor(
                            QKg.rearrange("p tt (h e) -> p tt h e", h=4),
                            QKg.rearrange("p tt (h e) -> p tt h e", h=4),
                            scl[:, :, :, None].to_broadcast((P, TT_N, 4, DH)),
                            mybir.AluOpType.mult,
                        )
                        # ---------- rotary (batched over all tt) ----------
                        for Xg in (Qg, Kg):
                            # rotating cols as [P, tt, hh=4(h,half), 16] (3 free
                            # dims - the ISA limit); cos64/sin64 tables are
                            # pre-tiled on host to the same [*, 64] layout
                            rot = Xg.rearrange(
                                "p tt (hh eh e) -> p tt hh eh e", hh=4, eh=2,
                            )[:, :, :, 0, :]  # [P, 8, 4, 16]
                            qsw = statp.tile([P, TT_N, 4, 16], f16, tag="qsw")
                            nc.vector.tensor_copy(
                                qsw[:, :, 0::2, :], rot[:, :, 1::2, :])
                            nc.vector.tensor_copy(
                                qsw[:, :, 1::2, :], rot[:, :, 0::2, :])
                            t1 = statp.tile([P, TT_N, 4, 16], f16, tag="t1")
                            nc.vector.tensor_tensor(
                                t1[:], rot,
                                cos_sb.rearrange(
                                    "p tt (hh e) -> p tt hh e", hh=4),
                                mybir.AluOpType.mult,
                            )
                            t2 = statp.tile([P, TT_N, 4, 16], f16, tag="t2")
                            nc.vector.tensor_tensor(
                                t2[:], qsw[:],
                                sin_sb.rearrange(
                                    "p tt (hh e) -> p tt hh e", hh=4),
                                mybir.AluOpType.mult,
                            )
                            nc.vector.tensor_tensor(
                                rot, t1[:], t2[:], mybir.AluOpType.add)
                        # ---------- transpose q,k via DRAM roundtrip ----------
                        qk_dr = qkdr.tile([T, 2 * P], f16, tag="qkdr")
                        nc.sync.dma_start(
                            out=qk_dr.rearrange("(tt p) r -> p tt r", p=P),
                            in_=QKg[:],
                        )
                        nc.sync.dma_start_transpose(
                            QT_sb[:, g, :], qk_dr[:, 0:P])
                        nc.sync.dma_start_transpose(
                            KT_sb[:, g, :], qk_dr[:, P : 2 * P])

                        # ---------- scores^T + exp (causal mask via ramp mm) ----
                        ET = etp.tile([P, KT_N, 2, T], f16, tag="et")
                        for kt in range(KT_N):
                            qlo = kt * P
                            pst = psST.tile([P, 2, T], f32, tag="st")
                            for hb in range(2):
                                lo, hi = hb * 64, hb * 64 + 64
                                for qh in range(2):
                                    qs = max(qh * 512, qlo)
                                    qe = (qh + 1) * 512
                                    if qs >= qe:
                                        continue
                                    diag = qs == qlo
                                    nc.tensor.matmul(
                                        pst[:, hb, ds(qs, qe - qs)],
                                        KT_sb[lo:hi, g, ts(kt, P)],
                                        QT_sb[lo:hi, g, ds(qs, qe - qs)],
                                        start=True, stop=not diag,
                                    )
                                    if diag:
                                        # additive -C*max(0, k-q) ramp kills
                                        # the upper triangle under exp
                                        nc.tensor.matmul(
                                            pst[:, hb, ds(qlo, P)],
                                            am_sb[:], bm_sb[:],
                                            start=False, stop=True,
                                        )
                            nc.scalar.activation(
                                ET[:, kt, :, ds(qlo, T - qlo)],
                                pst[:, :, ds(qlo, T - qlo)],
                                mybir.ActivationFunctionType.Exp,
                                scale=ATTN_SCALE,
                            )
                        # ---------- AV + divide (two q-tiles per psum bank) ----
                        for q2 in range(TT_N // 2):
                            pav = psAV.tile([P, 260], f32, tag="av")
                            first, last = None, None
                            mms = []
                            for sub in range(2):
                                qt = 2 * q2 + sub
                                for hb in range(2):
                                    for kt in range(qt + 1):
                                        mms.append((sub, qt, hb, kt))
                            for i, (sub, qt, hb, kt) in enumerate(mms):
                                nc.tensor.matmul(
                                    pav[:, ds(sub * 130 + hb * 65, 65)],
                                    ET[:, kt, hb, ts(qt, P)],
                                    vp[:, kt, 2 * g + hb, :],
                                    start=(i == 0),
                                    stop=(i == len(mms) - 1),
                                )
                            pavv = pav.rearrange("p (s h c) -> p s h c", s=2, h=2)
                            r = statp.tile([P, 2, 2], f32, tag="r")
                            nc.vector.reciprocal(r[:], pavv[:, :, :, DH : DH + 1])
                            nc.vector.tensor_tensor(
                                y16[:, ds(2 * q2, 2), gc].rearrange(
                                    "p s (h e) -> p s h e", h=2),
                                pavv[:, :, :, 0:DH],
                                r[:, :, :, None].to_broadcast((P, 2, 2, DH)),
                                mybir.AluOpType.mult,
                            )
                        # transpose this pair's y columns (jt == g)
                        y_dr = qkdr.tile([T, P], f16, tag="ydr")
                        nc.sync.dma_start(
                            out=y_dr.rearrange("(tt p) r -> p tt r", p=P),
                            in_=y16[:, :, gc],
                        )
                        nc.sync.dma_start_transpose(yT_sb[:, g, :], y_dr[:])
                # ================= output projection =====================
                with tc.tile_pool(name="outps", bufs=2, space="PSUM") as psF, \
                     tc.tile_pool(name="outstage", bufs=2) as osp:
                    for tt2 in range(TT_N // 2):
                        osb = osp.tile([P, 2, D], f32, tag="osb")
                        for sub in range(2):
                            tt = 2 * tt2 + sub
                            for ic in range(2):
                                po = psF.tile([P, 512], f32, tag="po")
                                for jt in range(JT_N):
                                    nc.tensor.matmul(
                                        po[:],
                                        yT_sb[:, jt, ts(tt, P)],
                                        wo_sb[:, jt, ds(ic * 512, 512)],
                                        start=(jt == 0), stop=(jt == JT_N - 1),
                                    )
                                nc.any.tensor_copy(
                                    out=osb[:, sub, ds(ic * 512, 512)], in_=po[:]
                                )
                        nc.sync.dma_start(
                            out=out_v[:, ds(2 * tt2, 2), :], in_=osb[:]
                        )

    split_sync_waits(nc)
    return nc


def make_core_inputs(x, qkvo_w, value_embeds, lambda_v):
    """Host-side prep: returns list of per-core input dicts (fp16)."""
    x = np.asarray(x)
    qkvo_w = np.asarray(qkvo_w)
    value_embeds = np.asarray(value_embeds)
    lambda_v = np.asarray(lambda_v)

    freq = (1.0 / 1024.0) ** np.linspace(0.0, 1.0, DH // 4, dtype=np.float32)
    theta = np.arange(T, dtype=np.float32)[:, None] * freq[None, :]  # [T, 16]
    cos = np.cos(theta).astype(np.float32)
    sin = np.sin(theta).astype(np.float32)
    # [T, 64] pre-tiled over (h,half) pairs: cos repeats, sin alternates sign
    cos64 = np.concatenate([cos, cos, cos, cos], axis=1).astype(np.float16)
    sin64 = np.concatenate([sin, -sin, sin, -sin], axis=1).astype(np.float16)
    # additive causal ramp mask: (amask.T @ bmask)[k, q] = -2000*max(0, k-q)
    jj = np.arange(P)
    amask_np = (jj[None, :] >= jj[:, None]).astype(np.float16)   # [j, k]
    bmask_np = (-2000.0 * (jj[:, None] > jj[None, :])).astype(np.float16)  # [j, q]

    in_maps = []
    for c in range(N_CORES):
        b, hh = c // 2, c % 2
        R = slice(hh * H8 * DH, (hh + 1) * H8 * DH)
        wq = qkvo_w[0][R].T  # [D, 512]
        wk = qkvo_w[1][R].T
        wv = (lambda_v[0] * qkvo_w[2][R]).T
        # [D, NG, 384]: per pair the 128 q cols, 128 k cols, 128 v cols
        wqkv = np.empty((D, NG, 384), dtype=np.float16)
        for g in range(NG):
            wqkv[:, g, 0:128] = wq[:, g * 128 : (g + 1) * 128]
            wqkv[:, g, 128:256] = wk[:, g * 128 : (g + 1) * 128]
            wqkv[:, g, 256:384] = wv[:, g * 128 : (g + 1) * 128]
        in_maps.append({
            "xT": np.ascontiguousarray(x[b].T).astype(np.float16),
            "wqkv": wqkv,
            "woT": np.ascontiguousarray(qkvo_w[3][:, R].T).astype(np.float16),
            "ve": (lambda_v[1] * value_embeds[:T, R]).astype(np.float16),
            "cosd": cos64,
            "sind": sin64,
            "amask": amask_np,
            "bmask": bmask_np,
        })
    return in_maps


_NC_CACHE = {}


def _get_nc(reps=1):
    if reps not in _NC_CACHE:
        _NC_CACHE[reps] = build_nc(reps)
    return _NC_CACHE[reps]


def kernel(x, qkvo_w, value_embeds, lambda_v):
    from concourse.bass_utils import run_bass_kernel_spmd

    nc = _get_nc()
    in_maps = make_core_inputs(x, qkvo_w, value_embeds, lambda_v)
    res = run_bass_kernel_spmd(nc, in_maps, list(range(N_CORES))).results
    out = np.empty((B, T, D), dtype=np.float32)
    for b in range(B):
        out[b] = res[2 * b]["out"] + res[2 * b + 1]["out"]
    return out
